# revision 45
# baseline (speedup 1.0000x reference)
"""Trainium2 Bass kernel for batched GCN message passing (nn_MLN_GCN).

Math per graph b (B=1024, data-parallel over 8 cores, 128 graphs/core):
    h0 = x[b,:,None] * embedding                  # [512, 64]
    h1 = relu(A @ (h0 @ W1) + b1)
    h2 = relu(A @ (h1 @ W2) + b2)
    logit = A @ (h2 @ W3) + b3                    # [512]
    out = [softmax(logit[:10]), sigmoid(logit[10:])]
with A[c, r] = sum_{e: col_e=c, row_e=r} norm_e the dense 512x512 normalized
adjacency shared across the batch.

Device structure (per core, 32 "quads" of 4 graphs = 2 pairs each):
  - layouts alternate feat-major [(b,f), n] / node-major [n, (b,f)] so the
    transform (contract features) and aggregation (contract nodes) matmuls
    chain with zero transposes.
  - layer-1 transform is free: h0 @ W1 == x[b,:,None] * (embedding @ W1).
  - layer-3 uses z2-stationary tiny matmuls so the [n, graph] output lands
    node-major directly (no per-pair DMA, no epilogue transposes).
  - aggregation matmuls run fp8e4 DoubleRow (2 rows/cycle); quantization
    scales fold into host-side ew1/w2/A copies, and the hidden activations
    are kept *scaled* (z~ = 1024*z) so the relu evacuations are pure
    bias+relu ops (the 1/1024 folds into the next layer's weights).
  - PSUM evacuations alternate Scalar/Vector per pair (different banks can
    be read in parallel); the x*EW1 multiply runs on the otherwise idle
    GpSimd engine; per-pair single-bank PSUM tiles on a 5-slot rotation
    keep every WAR dependency at least one pipeline step away.
  - emission is software-pipelined: PE order per step i is
    agg1(i), t3(i-2), agg2(i-1), t2(i) so every matmul's producer finished
    at least one step earlier.
  - ~3.5us of dummy matmuls at t=0 warm the PE HAM clock gate while the
    input DMAs land.
"""

import sys

import numpy as np

for _p in ("/opt/trn_rl_repo",):
    if _p not in sys.path:
        sys.path.append(_p)

B, NUM, D, H, E, MAIN = 1024, 512, 64, 64, 4096, 10
NCORES = 8
BC = B // NCORES        # graphs per core
NQ = BC // 4            # quads (4 graphs = 2 pairs) per core
NCH = NUM // 128        # node chunks of 128

USE_FP8 = True          # fp8e4 DoubleRow aggregation matmuls
Y1_ENGINE = "gpsimd"    # engine for the x*EW1 multiply: "gpsimd" | "vector"
SY = 64.0               # activation quantization scale (folded into ew1/w2)
SA = 16.0               # adjacency quantization scale

_CACHE = {}


def _build():
    key = "nc"
    if key in _CACHE:
        return _CACHE[key]

    import concourse.bacc as bacc
    import concourse.mybir as mybir
    from concourse import tile

    fp32 = mybir.dt.float32
    bf16 = mybir.dt.bfloat16
    fp8 = mybir.dt.float8e4
    ydt = fp8 if USE_FP8 else bf16
    AF = mybir.ActivationFunctionType
    AX = mybir.AxisListType
    ALU = mybir.AluOpType
    DR = mybir.MatmulPerfMode.DoubleRow

    nc = bacc.Bacc("TRN2", target_bir_lowering=False, debug=False)

    # All inputs are prepacked host-side into partition-major [128, ...]
    # layouts so every DMA is contiguous per partition.
    xt_d = nc.dram_tensor("xt", (128, NCH * BC), bf16, kind="ExternalInput")
    ew1_d = nc.dram_tensor("ew1", (128, NCH * H), bf16, kind="ExternalInput")
    at_d = nc.dram_tensor("a_t", (128, NCH * NUM), bf16, kind="ExternalInput")
    if USE_FP8:
        at8_d = nc.dram_tensor("a_t8", (128, 4 * NUM), fp8, kind="ExternalInput")
    w23_d = nc.dram_tensor("w23blk", (128, 130), bf16, kind="ExternalInput")
    bb_d = nc.dram_tensor("bblk", (128, 3), fp32, kind="ExternalInput")
    # Output staged in bf16 (host converts to fp32; ~1e-3 rel err, well
    # inside the 2e-2 gate) to halve the output DMA.
    out_d = nc.dram_tensor("out", (BC, NUM), bf16, kind="ExternalOutput")

    with tile.TileContext(nc) as tc:
        from contextlib import ExitStack

        with ExitStack() as ctx:
            const = ctx.enter_context(tc.tile_pool(name="const", bufs=1))
            y1p = ctx.enter_context(tc.tile_pool(name="y1", bufs=5))
            z1p = ctx.enter_context(tc.tile_pool(name="z1", bufs=3))
            y2p = ctx.enter_context(tc.tile_pool(name="y2", bufs=3))
            z2p = ctx.enter_context(tc.tile_pool(name="z2", bufs=3))
            zps = ctx.enter_context(tc.tile_pool(name="zps", bufs=5, space="PSUM"))
            tps = ctx.enter_context(tc.tile_pool(name="tps", bufs=2, space="PSUM"))
            t3psp = ctx.enter_context(tc.tile_pool(name="t3ps", bufs=1, space="PSUM"))

            xt = const.tile([128, NCH, BC], bf16)
            ew1 = const.tile([128, NCH, H], bf16)
            at = const.tile([128, NCH, NUM], bf16)
            if USE_FP8:
                at8 = const.tile([128, 2, 2, NUM], fp8)  # [p, cpair, klo/hi, n]
            w23 = const.tile([128, 130], bf16)
            w2 = w23[:, :128]
            w3 = w23[:, 128:]
            bb = const.tile([128, 3], fp32)
            b1t = bb[:, 0:1]
            b2t = bb[:, 1:2]
            b3t = bb[:, 2:3]
            y3t = const.tile([128, NCH, BC], bf16)   # [n, chunk, graph]
            outsb = const.tile([128, NUM], bf16)
            warm = const.tile([128, NUM], bf16)
            mx = const.tile([128, 1], fp32)
            nmx = const.tile([128, 1], fp32)
            ssum = const.tile([128, 1], fp32)
            rcp = const.tile([128, 1], fp32)
            es = const.tile([128, MAIN], fp32)

            nc.gpsimd.memset(warm[:], 0)

            # Prologue DMAs split across the three DMA trigger queues (sync,
            # scalar, gpsimd) so their first-byte latencies overlap; ordered
            # by when the pipeline needs each tensor (xt is split so the
            # first quads can start early; `at` is epilogue-only).
            nc.sync.dma_start(
                xt[:, :, :4],
                xt_d.ap().rearrange("p (c b) -> p c b", c=NCH)[:, :, :4],
            )
            nc.scalar.dma_start(ew1[:], ew1_d.ap().rearrange("p (c f) -> p c f", c=NCH))
            if USE_FP8:
                nc.scalar.dma_start(
                    at8[:], at8_d.ap().rearrange("p (c k n) -> p c k n", c=2, k=2)
                )
            nc.sync.dma_start(
                xt[:, :, 4:],
                xt_d.ap().rearrange("p (c b) -> p c b", c=NCH)[:, :, 4:],
            )
            nc.scalar.dma_start(w23[:], w23_d.ap()[:, :])
            nc.scalar.dma_start(bb[:], bb_d.ap()[:, :])
            nc.sync.dma_start(at[:], at_d.ap().rearrange("p (c n) -> p c n", c=NCH))

            # ---- engine warmup while the input DMAs land:
            # PE: ~3us of throwaway matmuls release the HAM clock gate;
            # GpSimd: a dummy tensor_tensor forces its ~6us ucode library
            # reload now instead of on the first real multiply; ScalarE: a
            # dummy Exp primes the activation table the epilogue needs.
            warmps = tps.tile([128, NUM], fp32, tag="t2")
            wg = const.tile([128, 8], bf16)
            if Y1_ENGINE == "gpsimd":
                nc.gpsimd.tensor_mul(wg[:], warm[:, :8], warm[:, 8:16])
            nc.scalar.activation(wg[:, :1], warm[:, :1], AF.Sigmoid)
            for _ in range(6):
                nc.tensor.matmul(warmps[:], warm[:, :128], warm[:], start=True, stop=True)

            def y1_mul(q):
                """y1[n, c, (pair,g), f] = x[g, n] * EW1[n, f] (scaled by SY).

                The first quad runs on Vector (GpSimd is still loading its
                ucode library) and as two per-pair ops, so agg1(0) can start
                after only pair A's multiply -- it heads the critical chain
                from the input DMAs to the first real matmul."""
                if Y1_ENGINE == "gpsimd" and q == 0:
                    halves = []
                    for p in range(2):
                        yh = y1p.tile([128, NCH, 2, H], ydt, tag="y1")
                        nc.vector.tensor_mul(
                            yh[:],
                            xt[:, :, 2 * p:2 * p + 2]
                            .unsqueeze(3)
                            .broadcast_to([128, NCH, 2, H]),
                            ew1[:].unsqueeze(2).broadcast_to([128, NCH, 2, H]),
                        )
                        halves.append(yh)
                    return halves
                eng = nc.vector if Y1_ENGINE != "gpsimd" else nc.gpsimd
                y1 = y1p.tile([128, NCH, 4, H], ydt, tag="y1")
                eng.tensor_mul(
                    y1[:],
                    xt[:, :, 4 * q:4 * q + 4].unsqueeze(3).broadcast_to([128, NCH, 4, H]),
                    ew1[:].unsqueeze(2).broadcast_to([128, NCH, 4, H]),
                )
                return y1

            def agg_pair(ps, ysel):
                """ps[:] = A-aggregation with lhsT slices from ysel."""
                if USE_FP8:
                    for cp in range(2):
                        nc.tensor.matmul(
                            ps[:], ysel(cp), at8[:, cp],
                            start=(cp == 0), stop=(cp == 1), perf_mode=DR,
                        )
                else:
                    for c in range(NCH):
                        nc.tensor.matmul(
                            ps[:], ysel(c), at[:, c, :],
                            start=(c == 0), stop=(c == NCH - 1),
                        )

            def relu_evac(p, zt, ps, bt):
                """z~[:, p] = relu(ps + b~); Scalar for pair 0, Vector for pair 1."""
                if p == 0:
                    nc.scalar.activation(zt[:, p], ps[:], AF.Relu, bias=bt)
                else:
                    nc.vector.tensor_scalar(
                        zt[:, p], ps[:], bt, 0.0, ALU.add, ALU.max
                    )

            def stage_agg1(q, y1):
                z1 = z1p.tile([128, 2, NUM], bf16, tag="z1")
                for p in range(2):
                    if isinstance(y1, list):
                        yp = y1[p]
                        ysel = (
                            (lambda cp: yp[:, 2 * cp:2 * cp + 2, :, :])
                            if USE_FP8
                            else (lambda c: yp[:, c, :, :])
                        )
                    elif USE_FP8:
                        ysel = lambda cp: y1[:, 2 * cp:2 * cp + 2, 2 * p:2 * p + 2, :]
                    else:
                        ysel = lambda c: y1[:, c, 2 * p:2 * p + 2, :]
                    z1ps = zps.tile([128, NUM], fp32, tag="z")
                    agg_pair(z1ps, ysel)
                    relu_evac(p, z1, z1ps, b1t)
                return z1

            def stage_t2(q, z1):
                """y2[n, pair, c, (g,o)] = z1 @ W2 (psum scaled to SY)."""
                y2 = y2p.tile([128, 2, NCH, 128], ydt, tag="y2")
                for p in range(2):
                    t2ps = tps.tile([128, NCH, 128], fp32, tag="t2")
                    for j in range(NCH):
                        nc.tensor.matmul(
                            t2ps[:, j, :], z1[:, p, j * 128:(j + 1) * 128], w2,
                            start=True, stop=True,
                        )
                    if p == 0:
                        nc.scalar.activation(y2[:, p], t2ps[:], AF.Copy)
                    else:
                        nc.vector.tensor_copy(y2[:, p], t2ps[:])
                return y2

            def stage_agg2(q, y2):
                z2 = z2p.tile([128, 2, NUM], bf16, tag="z2")
                for p in range(2):
                    z2ps = zps.tile([128, NUM], fp32, tag="z")
                    if USE_FP8:
                        agg_pair(z2ps, lambda cp: y2[:, p, 2 * cp:2 * cp + 2, :])
                    else:
                        agg_pair(z2ps, lambda c: y2[:, p, c, :])
                    relu_evac(p, z2, z2ps, b2t)
                return z2

            def stage_t3(q, z2):
                t3ps = t3psp.tile([128, 2, NCH, 2], fp32, tag="t3")
                for p in range(2):
                    for j in range(NCH):
                        nc.tensor.matmul(
                            t3ps[:, p, j, :], z2[:, p, j * 128:(j + 1) * 128], w3,
                            start=True, stop=True,
                        )
                for p in range(2):
                    nc.scalar.activation(
                        y3t[:, :, 4 * q + 2 * p:4 * q + 2 * p + 2],
                        t3ps[:, p],
                        AF.Copy,
                    )

            # ---- software-pipelined main loop over quads ----
            y1s, z1s, y2s, z2s = {}, {}, {}, {}
            y1s[0] = y1_mul(0)
            if NQ > 1:
                y1s[1] = y1_mul(1)
            for i in range(NQ + 2):
                if i + 2 < NQ:
                    y1s[i + 2] = y1_mul(i + 2)
                if i < NQ:
                    z1s[i] = stage_agg1(i, y1s.pop(i))
                if 0 <= i - 2:
                    stage_t3(i - 2, z2s.pop(i - 2))
                if 0 <= i - 1 < NQ:
                    z2s[i - 1] = stage_agg2(i - 1, y2s.pop(i - 1))
                if i < NQ:
                    y2s[i] = stage_t2(i, z1s.pop(i))

            # ---- epilogue: final aggregation of y3t, then softmax/sigmoid ----
            # A few filler matmuls keep the PE dense through the pipeline
            # drain so the HAM clock gate stays released for the final aggs.
            fillps = tps.tile([128, NUM], fp32, tag="t2")
            for _ in range(6):
                nc.tensor.matmul(fillps[:, :128], warm[:, :128], warm[:, :128],
                                 start=True, stop=True)
            lps = zps.tile([128, NUM], fp32, tag="z")
            for c in range(NCH):
                nc.tensor.matmul(
                    lps[:], y3t[:, c, :], at[:, c, :],
                    start=(c == 0), stop=(c == NCH - 1),
                )
            # Sigmoid first (its ACT table was primed at kernel start), then
            # the softmax Exp pays the only ACT_TABLE_LOAD of the tail.
            nc.scalar.activation(outsb[:, MAIN:], lps[:, MAIN:], AF.Sigmoid, bias=b3t)
            nc.vector.tensor_reduce(mx[:], lps[:, :MAIN], axis=AX.X, op=mybir.AluOpType.max)
            nc.sync.dma_start(out_d.ap()[:, MAIN:], outsb[:, MAIN:])
            nc.vector.tensor_scalar_mul(nmx[:], mx[:], -1.0)
            nc.scalar.activation(es[:], lps[:, :MAIN], AF.Exp, bias=nmx[:], accum_out=ssum[:])
            nc.vector.reciprocal(rcp[:], ssum[:])
            nc.vector.tensor_scalar_mul(outsb[:, :MAIN], es[:], rcp[:])

            nc.sync.dma_start(out_d.ap()[:, :MAIN], outsb[:, :MAIN])

    nc.compile()
    _CACHE[key] = nc
    return nc


def _prep_inputs(x, embedding, W1, b1, W2, b2, W3, b3, edge_row, edge_col):
    """Host prep: shard x over cores, build normalized adjacency + packed weights."""
    import ml_dtypes

    bf16 = ml_dtypes.bfloat16
    x = np.asarray(x, np.float32)
    embedding = np.asarray(embedding, np.float32)
    W1 = np.asarray(W1, np.float32)
    W2 = np.asarray(W2, np.float32)
    W3 = np.asarray(W3, np.float32)
    b1 = np.asarray(b1, np.float32)
    b2 = np.asarray(b2, np.float32)
    b3 = np.asarray(b3, np.float32)
    edge_row = np.asarray(edge_row)
    edge_col = np.asarray(edge_col)

    deg = np.zeros(NUM, np.float32)
    np.add.at(deg, edge_col, np.float32(1.0))
    dinv = np.where(deg > 0, (1.0 / np.sqrt(np.maximum(deg, 1.0))), 0.0).astype(np.float32)
    norm = (dinv[edge_row] * dinv[edge_col]).astype(np.float32)
    A = np.zeros((NUM, NUM), np.float32)
    np.add.at(A, (edge_col, edge_row), norm)
    a_t = np.ascontiguousarray(A.T)

    ew1 = (embedding @ W1).astype(np.float32)

    w2blk = np.zeros((128, 128), np.float32)
    w2blk[:H, :H] = W2
    w2blk[H:, H:] = W2
    w3blk = np.zeros((128, 2), np.float32)
    w3blk[:H, 0] = W3[:, 0]
    w3blk[H:, 1] = W3[:, 0]

    def pack(arr):
        """[NUM, F] -> partition-major [128, NCH*F] (chunk-of-128 rows -> dim1)."""
        F = arr.shape[1]
        return np.ascontiguousarray(
            arr.reshape(NCH, 128, F).transpose(1, 0, 2).reshape(128, NCH * F)
        )

    # Scale plumbing: y1/y2 are quantized with an extra SY, the adjacency
    # with SA; the hidden z~ activations stay scaled by S = SY*SA, which the
    # next layer's weight copy divides back out.
    S = SY * SA if USE_FP8 else 1.0
    w23blk = np.concatenate(
        [w2blk * np.float32((SY if USE_FP8 else 1.0) / S), w3blk * np.float32(1.0 / S)],
        axis=1,
    )
    bblk = np.stack(
        [np.tile(b1 * S, 2), np.tile(b2 * S, 2), np.full(128, b3[0], np.float32)],
        axis=1,
    )
    shared = dict(
        a_t=pack(a_t).astype(bf16),
        ew1=pack(ew1 * np.float32(SY if USE_FP8 else 1.0)).astype(bf16),
        w23blk=w23blk.astype(bf16),
        bblk=bblk.astype(np.float32),
    )
    if USE_FP8:
        import concourse.mybir as mybir

        fp8np = mybir.dt.np(mybir.dt.float8e4)
        shared["a_t8"] = pack(a_t * SA).astype(fp8np)

    in_maps = []
    for c in range(NCORES):
        xt = pack(np.ascontiguousarray(x[c * BC:(c + 1) * BC, :].T)).astype(bf16)
        in_maps.append(dict(xt=xt, **shared))
    return in_maps


def _run(inputs, trace=False):
    from concourse import bass_utils

    nc = _build()
    in_maps = _prep_inputs(**inputs)
    res = bass_utils.run_bass_kernel_spmd(
        nc, in_maps, core_ids=list(range(NCORES)), trace=trace,
    )
    out = np.concatenate([np.asarray(r["out"], np.float32) for r in res.results], axis=0)
    return out, res


def kernel(**inputs) -> np.ndarray:
    out, _ = _run(inputs, trace=False)
    return out


def kernel_traced(**inputs):
    """Returns (output, BassKernelResults with exec_time_ns/profile)."""
    return _run(inputs, trace=True)
